# revision 7
# baseline (speedup 1.0000x reference)
"""Multi-head causal self-attention on 8 trn2 NeuronCores.

Problem: x[2,2048,1024], 16 heads x 64 dim, causal softmax attention,
QKV/O projections with biases.

Sharding: core c handles batch b=c//4, head group g=c%4 (heads 4g..4g+3).
Each core computes its 4 heads' attention plus the partial O-projection;
the host sums the 4 partials per batch and adds bo.

Device layout (per core) is "transposed": host passes xT = x[b].T so the
feature dim lands on SBUF partitions.  QT/KT are [256, 2048] (head dims on
partitions), scores are computed transposed ST[k,q] = KT_kt^T @ QT, the
softmax denominator is accumulated for free via a ones-column appended to V
in the PV matmul, and the O-projection consumes ctxT directly.

Matmul dtypes: projections and O-proj in float32r (full PE rate, ~2e-4
rel err), PV in bf16 (errors average out over the long contraction).
"""
import sys

sys.path.insert(0, "/opt/trn_rl_repo")

import numpy as np

import concourse.bass as bass  # noqa: F401
import concourse.tile as tile
from concourse import bacc
from concourse import mybir

F32 = mybir.dt.float32
F32R = mybir.dt.float32r
BF16 = mybir.dt.bfloat16

T = 2048          # sequence length
C = 1024          # model dim
HG = 4            # heads per core
HD = 64           # head dim
DG = HG * HD      # 256, projected dims per core
NF = C // 128     # 8 feature chunks
NT = T // 128     # 16 token tiles
NQ = T // 512     # 4 q-chunks
SCALE = 0.125     # 1/sqrt(64)


def build_kernel():
    nc = bacc.Bacc("TRN2")
    xT_d = nc.dram_tensor("xT", [C, T], F32R, kind="ExternalInput").ap()
    wq_d = nc.dram_tensor("wq", [C, DG], F32R, kind="ExternalInput").ap()
    wk_d = nc.dram_tensor("wk", [C, DG], F32R, kind="ExternalInput").ap()
    wv_d = nc.dram_tensor("wv", [C, DG], F32R, kind="ExternalInput").ap()
    # wo: [4, 64, 1024] per-head rows of Wo
    wo_d = nc.dram_tensor("wo", [HG, HD, C], F32R, kind="ExternalInput").ap()
    bq_d = nc.dram_tensor("bq", [128, 2], F32, kind="ExternalInput").ap()
    bk_d = nc.dram_tensor("bk", [128, 2], F32, kind="ExternalInput").ap()
    # bv broadcast to partitions on host: [128, 256]
    bv_d = nc.dram_tensor("bv", [128, DG], F32, kind="ExternalInput").ap()
    out_d = nc.dram_tensor("out", [T, C], F32, kind="ExternalOutput").ap()

    with tile.TileContext(nc) as tc:
        with tc.tile_pool(name="persist", bufs=1) as pp:
            qt = pp.tile([128, 2, T], F32R, name="qt")     # QT d'-chunks
            kt = pp.tile([128, 2, T], F32R, name="kt")     # KT d'-chunks
            vsb = pp.tile([128, NT, HG, HD + 1], BF16, name="vsb")  # [V|1]
            wo_sb = pp.tile([HD, HG, C], F32R, name="wo_sb")
            bq_sb = pp.tile([128, 2], F32, name="bq_sb")
            bk_sb = pp.tile([128, 2], F32, name="bk_sb")
            bv_sb = pp.tile([128, DG], F32, name="bv_sb")
            masks = pp.tile([128, 4, 512], BF16, name="masks")

            nc.sync.dma_start(bq_sb[:], bq_d)
            nc.sync.dma_start(bk_sb[:], bk_d)
            nc.sync.dma_start(bv_sb[:], bv_d)
            for h in range(HG):
                nc.sync.dma_start(wo_sb[:, h, :], wo_d[h])

            # ones column of V_ext
            nc.gpsimd.memset(vsb[:, :, :, HD:HD + 1], 1.0)

            # causal masks M_r [128, 512] bf16: keep (1.0) iff q - k - 128r >= 0
            for r in range(4):
                nc.gpsimd.memset(masks[:, r, :], 1.0)
                nc.gpsimd.affine_select(
                    out=masks[:, r, :],
                    in_=masks[:, r, :],
                    compare_op=mybir.AluOpType.is_ge,
                    fill=0.0,
                    base=-128 * r,
                    pattern=[[1, 512]],
                    channel_multiplier=-1,
                )

            # ---------------- Phase A: projections ----------------
            _sid_proj, _ = nc.enter_named_scope("proj", False)
            with tc.tile_pool(name="xtp", bufs=1) as xtp, \
                 tc.tile_pool(name="wp", bufs=2) as wp, \
                 tc.tile_pool(name="pjp", bufs=2, space="PSUM") as pjp:
                xt = xtp.tile([128, NF, T], F32R, name="xt")
                for f in range(NF):
                    nc.sync.dma_start(xt[:, f, :], xT_d[128 * f:128 * (f + 1), :])

                w_srcs = {"q": wq_d, "k": wk_d, "v": wv_d}
                w_tiles = {}

                def load_w(which):
                    w_tiles[which] = wp.tile([128, NF, DG], F32R,
                                             name=f"w{which}", tag="w")
                    nc.sync.dma_start(
                        w_tiles[which][:],
                        w_srcs[which].rearrange("(f p) d -> p f d", p=128))

                load_w("q")
                load_w("k")
                # QT / KT: psum [128, 2048] per d'-chunk, accumulate over f
                for dst, wkey, b_sb in ((qt, "q", bq_sb), (kt, "k", bk_sb)):
                    w_sb = w_tiles[wkey]
                    for dc in range(2):
                        ps = pjp.tile([128, T], F32, name="pjqk", tag="pj")
                        for f in range(NF):
                            lhsT = w_sb[:, f, 128 * dc:128 * (dc + 1)]
                            for t4 in range(NQ):
                                nc.tensor.matmul(
                                    ps[:, 512 * t4:512 * (t4 + 1)],
                                    lhsT,
                                    xt[:, f, 512 * t4:512 * (t4 + 1)],
                                    start=(f == 0), stop=(f == NF - 1),
                                )
                        nc.scalar.activation(
                            dst[:, dc, :], ps[:],
                            mybir.ActivationFunctionType.Identity,
                            bias=b_sb[:, dc:dc + 1])

                load_w("v")
                wv_sb = w_tiles["v"]
                # V: per t-tile, accumulate over f; out [128, 256] + bias, bf16
                for t in range(NT):
                    ps = pjp.tile([128, DG], F32, name="pjv", tag="pj")
                    for f in range(NF):
                        nc.tensor.matmul(
                            ps[:],
                            xt[:, f, 128 * t:128 * (t + 1)],
                            wv_sb[:, f, :],
                            start=(f == 0), stop=(f == NF - 1),
                        )
                    # V_sb[:, t, h, 0:64] = ps[:, 64h:64h+64] + bv
                    nc.vector.tensor_tensor(
                        vsb[:, t, :, 0:HD],
                        ps[:].rearrange("p (h d) -> p h d", h=HG),
                        bv_sb[:].rearrange("p (h d) -> p h d", h=HG),
                        mybir.AluOpType.add)

            nc.leave_named_scope("proj", _sid_proj, False)
            # ---------------- Phase B: attention + O-proj ----------------
            with tc.tile_pool(name="ctxp", bufs=1) as ctxp, \
                 tc.tile_pool(name="pp2", bufs=6) as pbuf, \
                 tc.tile_pool(name="outp", bufs=3) as outp, \
                 tc.tile_pool(name="dnp", bufs=4) as dnp, \
                 tc.tile_pool(name="drp", bufs=4, space="DRAM") as drp, \
                 tc.tile_pool(name="sps", bufs=4, space="PSUM") as sps, \
                 tc.tile_pool(name="cps", bufs=2, space="PSUM") as cps, \
                 tc.tile_pool(name="ops", bufs=2, space="PSUM") as ops:

                ctxt = [ctxp.tile([HD, T], F32R, name=f"ctxt{h}")
                        for h in range(HG)]

                for qc in range(NQ):
                    _sid_a, _ = nc.enter_named_scope(f"attn{qc}", False)
                    nkt = 4 * qc + 4   # causal: k-tiles 0 .. 4qc+3
                    for pair in range(2):
                        heads = (2 * pair, 2 * pair + 1)
                        dc = pair
                        cps_t = {h: cps.tile([HD + 1, 512], F32,
                                             name=f"cps{h}", tag="cps")
                                 for h in heads}
                        for k in range(nkt):
                            r = k - 4 * qc   # >=0 on diagonal-crossing tiles
                            ptiles = {}
                            for j, h in enumerate(heads):
                                hp = 64 * j   # partition offset within chunk
                                s_ps = sps.tile([128, 512], F32, name="s_ps",
                                                tag="s")
                                lk = kt[hp:hp + 64, dc, 128 * k:128 * (k + 1)]
                                lq = qt[hp:hp + 64, dc, 512 * qc:512 * (qc + 1)]
                                nc.tensor.matmul(s_ps[:], lk, lq,
                                                 start=True, stop=True)
                                # exp(S/8) -> bf16 P; cols < 128r skipped
                                p = pbuf.tile([128, 512], BF16, name="p",
                                              tag="p")
                                c0 = 128 * r if r > 0 else 0
                                if c0 > 0:
                                    nc.gpsimd.memset(p[:, 0:c0], 0.0)
                                nc.scalar.activation(
                                    p[:, c0:], s_ps[:, c0:],
                                    mybir.ActivationFunctionType.Exp,
                                    scale=SCALE)
                                if r >= 0:
                                    # mask the diagonal 128x128 block
                                    nc.vector.tensor_tensor(
                                        p[:, c0:c0 + 128],
                                        p[:, c0:c0 + 128],
                                        masks[:, r, c0:c0 + 128],
                                        mybir.AluOpType.mult)
                                ptiles[h] = p
                            for j, h in enumerate(heads):
                                nc.tensor.matmul(
                                    cps_t[h][:],
                                    vsb[:, k, h, :],
                                    ptiles[h][:],
                                    start=(k == 0), stop=(k == nkt - 1),
                                )
                        # denominators + normalize + copy out ctxT
                        for h in heads:
                            dn = dnp.tile([65, 512], F32, name="dn", tag="dn")
                            nc.scalar.copy(dn[:, :], cps_t[h][:, :])
                            dn2 = dnp.tile([65, 512], F32, name="dn2",
                                           tag="dn2")
                            nc.vector.reciprocal_approx_fast(
                                out=dn2[:, 0:512], in_=dn[:, 0:512])
                            dr = drp.tile([1, 512], F32, name="dr", tag="dr")
                            nc.sync.dma_start(dr[:], dn2[64:65, :])
                            bc = dnp.tile([HD, 512], F32, name="bc", tag="bc")
                            nc.sync.dma_start(
                                bc[:], dr[:].to_broadcast((HD, 512)))
                            nc.vector.tensor_tensor(
                                ctxt[h][:, 512 * qc:512 * (qc + 1)],
                                cps_t[h][0:HD, :], bc[:],
                                mybir.AluOpType.mult)

                    nc.leave_named_scope(f"attn{qc}", _sid_a, False)
                    _sid_o, _ = nc.enter_named_scope(f"oproj{qc}", False)
                    # O-projection for this q-chunk's 4 token tiles
                    for tt in range(4):
                        t0 = 512 * qc + 128 * tt
                        for c2 in range(2):
                            o_ps = ops.tile([128, 512], F32, name="o_ps",
                                            tag="o")
                            for h in range(HG):
                                nc.tensor.matmul(
                                    o_ps[:],
                                    ctxt[h][:, t0:t0 + 128],
                                    wo_sb[:, h, 512 * c2:512 * (c2 + 1)],
                                    start=(h == 0), stop=(h == HG - 1),
                                )
                            o_sb = outp.tile([128, 512], F32, name="o_sb",
                                             tag="osb")
                            nc.vector.tensor_copy(o_sb[:], o_ps[:])
                            nc.sync.dma_start(
                                out_d[t0:t0 + 128, 512 * c2:512 * (c2 + 1)],
                                o_sb[:])
                    nc.leave_named_scope(f"oproj{qc}", _sid_o, False)

    nc.compile()
    return nc


_NC_CACHE = None


def _get_nc():
    global _NC_CACHE
    if _NC_CACHE is None:
        _NC_CACHE = build_kernel()
    return _NC_CACHE


def make_in_maps(x, Wq, bq, Wk, bk, Wv, bv, Wo, bo):
    in_maps = []
    for c in range(8):
        b, g = c // 4, c % 4
        sl = slice(256 * g, 256 * (g + 1))
        bqg = np.ascontiguousarray(bq[sl].reshape(2, 128).T)
        bkg = np.ascontiguousarray(bk[sl].reshape(2, 128).T)
        bvg = np.ascontiguousarray(np.tile(bv[sl][None, :], (128, 1)))
        in_maps.append({
            "xT": np.ascontiguousarray(x[b].T),
            "wq": np.ascontiguousarray(Wq[:, sl]),
            "wk": np.ascontiguousarray(Wk[:, sl]),
            "wv": np.ascontiguousarray(Wv[:, sl]),
            "wo": np.ascontiguousarray(Wo[sl, :].reshape(HG, HD, C)),
            "bq": bqg.astype(np.float32),
            "bk": bkg.astype(np.float32),
            "bv": bvg.astype(np.float32),
        })
    return in_maps


def combine_outputs(results, bo):
    out = np.empty((2, T, C), np.float32)
    for b in range(2):
        acc = results[4 * b]["out"].astype(np.float32).copy()
        for g in range(1, 4):
            acc += results[4 * b + g]["out"]
        out[b] = acc + bo[None, :]
    return out


def kernel(**inputs):
    from concourse.bass_utils import run_bass_kernel_spmd
    args = {k: np.asarray(v, np.float32) for k, v in inputs.items()}
    nc = _get_nc()
    in_maps = make_in_maps(
        args["x"], args["Wq"], args["bq"], args["Wk"], args["bk"],
        args["Wv"], args["bv"], args["Wo"], args["bo"])
    res = run_bass_kernel_spmd(nc, in_maps, core_ids=list(range(8)))
    return combine_outputs(res.results, args["bo"])


# revision 8
# speedup vs baseline: 1.2015x; 1.2015x over previous
"""Multi-head causal self-attention on 8 trn2 NeuronCores.

Problem: x[2,2048,1024], 16 heads x 64 dim, causal softmax attention,
QKV/O projections with biases.

Sharding: core c handles batch b=c//4, head group g=c%4 (heads 4g..4g+3).
Each core computes its 4 heads' attention plus the partial O-projection;
the host sums the 4 partials per batch and adds bo.

Device layout (per core) is "transposed": host passes xT = x[b].T so the
feature dim lands on SBUF partitions.  QT/KT are [256, 2048] (head dims on
partitions), scores are computed transposed ST[k,q] = KT_kt^T @ QT, the
softmax denominator is accumulated for free via a ones-column appended to V
in the PV matmul, and the O-projection consumes ctxT directly.

Matmul dtypes: projections and O-proj in float32r (full PE rate, ~2e-4
rel err), PV in bf16 (errors average out over the long contraction).
"""
import sys

sys.path.insert(0, "/opt/trn_rl_repo")

import numpy as np

import concourse.bass as bass  # noqa: F401
import concourse.tile as tile
from concourse import bacc
from concourse import mybir

F32 = mybir.dt.float32
F32R = mybir.dt.float32r
BF16 = mybir.dt.bfloat16

T = 2048          # sequence length
C = 1024          # model dim
HG = 4            # heads per core
HD = 64           # head dim
DG = HG * HD      # 256, projected dims per core
NF = C // 128     # 8 feature chunks
NT = T // 128     # 16 token tiles
NQ = T // 512     # 4 q-chunks
SCALE = 0.125     # 1/sqrt(64)


def build_kernel():
    nc = bacc.Bacc("TRN2")
    xT_d = nc.dram_tensor("xT", [C, T], F32R, kind="ExternalInput").ap()
    wq_d = nc.dram_tensor("wq", [C, DG], F32R, kind="ExternalInput").ap()
    wk_d = nc.dram_tensor("wk", [C, DG], F32R, kind="ExternalInput").ap()
    wv_d = nc.dram_tensor("wv", [C, DG], F32R, kind="ExternalInput").ap()
    # wo: [4, 64, 1024] per-head rows of Wo
    wo_d = nc.dram_tensor("wo", [HG, HD, C], F32R, kind="ExternalInput").ap()
    bq_d = nc.dram_tensor("bq", [128, 2], F32, kind="ExternalInput").ap()
    bk_d = nc.dram_tensor("bk", [128, 2], F32, kind="ExternalInput").ap()
    # bv broadcast to partitions on host: [128, 256]
    bv_d = nc.dram_tensor("bv", [128, DG], F32, kind="ExternalInput").ap()
    out_d = nc.dram_tensor("out", [T, C], F32, kind="ExternalOutput").ap()

    with tile.TileContext(nc) as tc:
        with tc.tile_pool(name="persist", bufs=1) as pp:
            qt = pp.tile([128, 2, T], F32R, name="qt")     # QT d'-chunks
            kt = pp.tile([128, 2, T], F32R, name="kt")     # KT d'-chunks
            vsb = pp.tile([128, NT, HG, HD + 1], BF16, name="vsb")  # [V|1]
            wo_sb = pp.tile([HD, HG, C], F32R, name="wo_sb")
            bq_sb = pp.tile([128, 2], F32, name="bq_sb")
            bk_sb = pp.tile([128, 2], F32, name="bk_sb")
            bv_sb = pp.tile([128, DG], F32, name="bv_sb")
            masks = pp.tile([128, 4, 512], BF16, name="masks")

            nc.sync.dma_start(bq_sb[:], bq_d)
            nc.sync.dma_start(bk_sb[:], bk_d)
            nc.sync.dma_start(bv_sb[:], bv_d)
            for h in range(HG):
                nc.sync.dma_start(wo_sb[:, h, :], wo_d[h])

            # ones column of V_ext
            nc.gpsimd.memset(vsb[:, :, :, HD:HD + 1], 1.0)

            # causal masks M_r [128, 512] bf16: keep (1.0) iff q - k - 128r >= 0
            for r in range(4):
                nc.gpsimd.memset(masks[:, r, :], 1.0)
                nc.gpsimd.affine_select(
                    out=masks[:, r, :],
                    in_=masks[:, r, :],
                    compare_op=mybir.AluOpType.is_ge,
                    fill=0.0,
                    base=-128 * r,
                    pattern=[[1, 512]],
                    channel_multiplier=-1,
                )

            # ---------------- Phase A: projections ----------------
            _sid_proj, _ = nc.enter_named_scope("proj", False)
            with tc.tile_pool(name="xtp", bufs=1) as xtp, \
                 tc.tile_pool(name="wp", bufs=2) as wp, \
                 tc.tile_pool(name="pjp", bufs=2, space="PSUM") as pjp:
                xt = xtp.tile([128, NF, T], F32R, name="xt")
                for f in range(NF):
                    nc.sync.dma_start(xt[:, f, :], xT_d[128 * f:128 * (f + 1), :])

                w_srcs = {"q": wq_d, "k": wk_d, "v": wv_d}
                w_tiles = {}

                def load_w(which):
                    w_tiles[which] = wp.tile([128, NF, DG], F32R,
                                             name=f"w{which}", tag="w")
                    nc.sync.dma_start(
                        w_tiles[which][:],
                        w_srcs[which].rearrange("(f p) d -> p f d", p=128))

                load_w("q")
                load_w("k")
                # QT / KT: psum [128, 2048] per d'-chunk, accumulate over f
                for dst, wkey, b_sb in ((qt, "q", bq_sb), (kt, "k", bk_sb)):
                    w_sb = w_tiles[wkey]
                    for dc in range(2):
                        ps = pjp.tile([128, T], F32, name="pjqk", tag="pj")
                        for f in range(NF):
                            lhsT = w_sb[:, f, 128 * dc:128 * (dc + 1)]
                            for t4 in range(NQ):
                                nc.tensor.matmul(
                                    ps[:, 512 * t4:512 * (t4 + 1)],
                                    lhsT,
                                    xt[:, f, 512 * t4:512 * (t4 + 1)],
                                    start=(f == 0), stop=(f == NF - 1),
                                )
                        nc.scalar.activation(
                            dst[:, dc, :], ps[:],
                            mybir.ActivationFunctionType.Identity,
                            bias=b_sb[:, dc:dc + 1])

                load_w("v")
                wv_sb = w_tiles["v"]
                # V: per t-tile, accumulate over f; out [128, 256] + bias, bf16
                for t in range(NT):
                    ps = pjp.tile([128, DG], F32, name="pjv", tag="pj")
                    for f in range(NF):
                        nc.tensor.matmul(
                            ps[:],
                            xt[:, f, 128 * t:128 * (t + 1)],
                            wv_sb[:, f, :],
                            start=(f == 0), stop=(f == NF - 1),
                        )
                    # V_sb[:, t, h, 0:64] = ps[:, 64h:64h+64] + bv
                    nc.vector.tensor_tensor(
                        vsb[:, t, :, 0:HD],
                        ps[:].rearrange("p (h d) -> p h d", h=HG),
                        bv_sb[:].rearrange("p (h d) -> p h d", h=HG),
                        mybir.AluOpType.add)

            nc.leave_named_scope("proj", _sid_proj, False)
            # ---------------- Phase B: attention + O-proj ----------------
            with tc.tile_pool(name="ctxp", bufs=1) as ctxp, \
                 tc.tile_pool(name="pp2", bufs=6) as pbuf, \
                 tc.tile_pool(name="outp", bufs=3) as outp, \
                 tc.tile_pool(name="dnp", bufs=4) as dnp, \
                 tc.tile_pool(name="drp", bufs=4, space="DRAM") as drp, \
                 tc.tile_pool(name="sps", bufs=2, space="PSUM") as sps, \
                 tc.tile_pool(name="cps", bufs=2, space="PSUM") as cps, \
                 tc.tile_pool(name="ops", bufs=2, space="PSUM") as ops:

                ctxt = [ctxp.tile([HD, T], F32R, name=f"ctxt{h}")
                        for h in range(HG)]

                for qc in range(NQ):
                    _sid_a, _ = nc.enter_named_scope(f"attn{qc}", False)
                    nkt = 4 * qc + 4   # causal: k-tiles 0 .. 4qc+3
                    for pair in range(2):
                        heads = (2 * pair, 2 * pair + 1)
                        dc = pair
                        cps_t = {h: cps.tile([HD + 1, 512], F32,
                                             name=f"cps{h}", tag="cps")
                                 for h in heads}
                        for k in range(nkt):
                            r = k - 4 * qc   # >=0 on diagonal-crossing tiles
                            # paired S matmuls into one 2-bank psum tile
                            s_ps = sps.tile([128, 1024], F32, name="s_ps",
                                            tag="s")
                            for j, h in enumerate(heads):
                                hp = 64 * j   # partition offset within chunk
                                lk = kt[hp:hp + 64, dc, 128 * k:128 * (k + 1)]
                                lq = qt[hp:hp + 64, dc, 512 * qc:512 * (qc + 1)]
                                nc.tensor.matmul(
                                    s_ps[:, 512 * j:512 * (j + 1)], lk, lq,
                                    start=True, stop=True)
                            # one exp over both heads' scores
                            p = pbuf.tile([128, 1024], BF16, name="p", tag="p")
                            nc.scalar.activation(
                                p[:], s_ps[:],
                                mybir.ActivationFunctionType.Exp, scale=SCALE)
                            if r >= 0:
                                # zero left-of-diagonal + mask the triangle
                                c1 = 128 * (r + 1)
                                for j in range(2):
                                    nc.vector.tensor_tensor(
                                        p[:, 512 * j:512 * j + c1],
                                        p[:, 512 * j:512 * j + c1],
                                        masks[:, r, 0:c1],
                                        mybir.AluOpType.mult)
                            for j, h in enumerate(heads):
                                nc.tensor.matmul(
                                    cps_t[h][:],
                                    vsb[:, k, h, :],
                                    p[:, 512 * j:512 * (j + 1)],
                                    start=(k == 0), stop=(k == nkt - 1),
                                )
                        # denominators + normalize + copy out ctxT
                        for h in heads:
                            dn = dnp.tile([65, 512], F32, name="dn", tag="dn")
                            nc.scalar.copy(dn[:, :], cps_t[h][:, :])
                            dn2 = dnp.tile([65, 512], F32, name="dn2",
                                           tag="dn2")
                            nc.vector.reciprocal_approx_fast(
                                out=dn2[:, 0:512], in_=dn[:, 0:512])
                            dr = drp.tile([1, 512], F32, name="dr", tag="dr")
                            nc.gpsimd.dma_start(dr[:], dn2[64:65, :])
                            bc = dnp.tile([HD, 512], F32, name="bc", tag="bc")
                            nc.gpsimd.dma_start(
                                bc[:], dr[:].to_broadcast((HD, 512)))
                            nc.vector.tensor_tensor(
                                ctxt[h][:, 512 * qc:512 * (qc + 1)],
                                cps_t[h][0:HD, :], bc[:],
                                mybir.AluOpType.mult)

                    nc.leave_named_scope(f"attn{qc}", _sid_a, False)
                    _sid_o, _ = nc.enter_named_scope(f"oproj{qc}", False)
                    # O-projection for this q-chunk's 4 token tiles
                    for tt in range(4):
                        t0 = 512 * qc + 128 * tt
                        for c2 in range(2):
                            o_ps = ops.tile([128, 512], F32, name="o_ps",
                                            tag="o")
                            for h in range(HG):
                                nc.tensor.matmul(
                                    o_ps[:],
                                    ctxt[h][:, t0:t0 + 128],
                                    wo_sb[:, h, 512 * c2:512 * (c2 + 1)],
                                    start=(h == 0), stop=(h == HG - 1),
                                )
                            o_sb = outp.tile([128, 512], F32, name="o_sb",
                                             tag="osb")
                            nc.vector.tensor_copy(o_sb[:], o_ps[:])
                            nc.sync.dma_start(
                                out_d[t0:t0 + 128, 512 * c2:512 * (c2 + 1)],
                                o_sb[:])
                    nc.leave_named_scope(f"oproj{qc}", _sid_o, False)

    nc.compile()
    return nc


_NC_CACHE = None


def _get_nc():
    global _NC_CACHE
    if _NC_CACHE is None:
        _NC_CACHE = build_kernel()
    return _NC_CACHE


def make_in_maps(x, Wq, bq, Wk, bk, Wv, bv, Wo, bo):
    in_maps = []
    for c in range(8):
        b, g = c // 4, c % 4
        sl = slice(256 * g, 256 * (g + 1))
        bqg = np.ascontiguousarray(bq[sl].reshape(2, 128).T)
        bkg = np.ascontiguousarray(bk[sl].reshape(2, 128).T)
        bvg = np.ascontiguousarray(np.tile(bv[sl][None, :], (128, 1)))
        in_maps.append({
            "xT": np.ascontiguousarray(x[b].T),
            "wq": np.ascontiguousarray(Wq[:, sl]),
            "wk": np.ascontiguousarray(Wk[:, sl]),
            "wv": np.ascontiguousarray(Wv[:, sl]),
            "wo": np.ascontiguousarray(Wo[sl, :].reshape(HG, HD, C)),
            "bq": bqg.astype(np.float32),
            "bk": bkg.astype(np.float32),
            "bv": bvg.astype(np.float32),
        })
    return in_maps


def combine_outputs(results, bo):
    out = np.empty((2, T, C), np.float32)
    for b in range(2):
        acc = results[4 * b]["out"].astype(np.float32).copy()
        for g in range(1, 4):
            acc += results[4 * b + g]["out"]
        out[b] = acc + bo[None, :]
    return out


def kernel(**inputs):
    from concourse.bass_utils import run_bass_kernel_spmd
    args = {k: np.asarray(v, np.float32) for k, v in inputs.items()}
    nc = _get_nc()
    in_maps = make_in_maps(
        args["x"], args["Wq"], args["bq"], args["Wk"], args["bk"],
        args["Wv"], args["bv"], args["Wo"], args["bo"])
    res = run_bass_kernel_spmd(nc, in_maps, core_ids=list(range(8)))
    return combine_outputs(res.results, args["bo"])
